# revision 12
# baseline (speedup 1.0000x reference)
import os
import sys

import numpy as np

sys.path.insert(0, "/opt/trn_rl_repo")

# Problem constants (nn_AdditiveAttention): hardcoded per spec.
B, NQ, NK, D, DV, H = 4, 512, 512, 512, 512, 128
NCORES = 8
QPC = NQ // NCORES  # queries contributed by each batch to each core (64)
SMAX = 9216         # max s/t pipeline tile free dim (per partition)

LAST_EXEC_NS = None
LAST_RESULT = {}


def _plan(valid_lens):
    L = [int(x) for x in np.asarray(valid_lens).reshape(-1)]
    L2 = [min(NK, -(-l // 2) * 2) for l in L]       # add/tanh/scores extent
    KPV = [min(NK, -(-l // 128) * 128) for l in L]  # PV (128-aligned) extent
    CH = {}
    for b in range(B):
        c = 32
        while c * L2[b] > SMAX:
            c //= 2
        CH[b] = c
    return L, L2, KPV, CH


def _pi(j):
    """Ping-pong permutation within a 64-query group: query j -> row offset.
    Alternates 32-row col-groups so consecutive score matmuls hit different
    PE column groups and overlap."""
    return (j % 2) * 32 + j // 2


def _build_program(L, L2, KPV, CH):
    """Build the SPMD Bass program. All cores run this one program;
    per-core data differences come only through in_maps."""
    import concourse.bacc as bacc
    import concourse.mybir as mybir
    from concourse.tile import TileContext

    f32 = mybir.dt.float32
    bf16 = mybir.dt.bfloat16
    OFF2 = np.concatenate([[0], np.cumsum(L2)]).astype(int)
    OFFV = np.concatenate([[0], np.cumsum(KPV)]).astype(int)
    KSUM2 = int(OFF2[-1])
    KSUMV = int(OFFV[-1])
    NQL = B * QPC  # local queries per core (256)

    nc = bacc.Bacc("TRN2", target_bir_lowering=False, debug=False)

    qt_d = nc.dram_tensor("qt", [D, NQL], bf16, kind="ExternalInput")
    kt_d = nc.dram_tensor("kt", [D, KSUM2], bf16, kind="ExternalInput")
    v_d = nc.dram_tensor("v", [KSUMV, DV], bf16, kind="ExternalInput")
    wq_d = nc.dram_tensor("wq", [D, H], bf16, kind="ExternalInput")
    wk_d = nc.dram_tensor("wk", [D, H], bf16, kind="ExternalInput")
    oneh_d = nc.dram_tensor("oneh", [H, 32 * 32], bf16, kind="ExternalInput")
    eye_d = nc.dram_tensor("eye", [128, QPC], bf16, kind="ExternalInput")
    out_d = nc.dram_tensor("out", [NQL, DV], f32, kind="ExternalOutput")

    Tanh = mybir.ActivationFunctionType.Tanh
    Exp = mybir.ActivationFunctionType.Exp
    Copy = mybir.ActivationFunctionType.Copy
    AX = mybir.AxisListType.X

    with TileContext(nc) as tc:
        with (
            tc.tile_pool(name="const", bufs=1) as cpool,
            tc.tile_pool(name="proj", bufs=1) as projpool,
            tc.tile_pool(name="s", bufs=3) as spool,
            tc.tile_pool(name="t", bufs=3) as tpool,
            tc.tile_pool(name="p", bufs=2) as ppool,
            tc.tile_pool(name="stat", bufs=4) as statpool,
            tc.tile_pool(name="osb", bufs=2) as opool,
            tc.tile_pool(name="pps", bufs=2, space="PSUM") as projps,
            tc.tile_pool(name="sps", bufs=2, space="PSUM") as scorps,
            tc.tile_pool(name="ops", bufs=2, space="PSUM") as ops,
            tc.tile_pool(name="tps", bufs=2, space="PSUM") as tps,
        ):
            # ---- load constants
            qt_sb = []
            kt_sb = []
            wq_sb = []
            wk_sb = []
            for i in range(4):
                t = cpool.tile([128, NQL], bf16, tag=f"qt{i}")
                nc.sync.dma_start(t[:], qt_d.rearrange("(n p) m -> n p m", p=128)[i])
                qt_sb.append(t)
                t = cpool.tile([128, KSUM2], bf16, tag=f"kt{i}")
                nc.sync.dma_start(t[:], kt_d.rearrange("(n p) m -> n p m", p=128)[i])
                kt_sb.append(t)
                t = cpool.tile([128, H], bf16, tag=f"wq{i}")
                nc.sync.dma_start(t[:], wq_d.rearrange("(n p) m -> n p m", p=128)[i])
                wq_sb.append(t)
                t = cpool.tile([128, H], bf16, tag=f"wk{i}")
                nc.sync.dma_start(t[:], wk_d.rearrange("(n p) m -> n p m", p=128)[i])
                wk_sb.append(t)
            v_sb = []
            for i in range(KSUMV // 128):
                t = cpool.tile([128, DV], bf16, tag=f"v{i}")
                nc.sync.dma_start(t[:], v_d.rearrange("(n p) m -> n p m", p=128)[i])
                v_sb.append(t)
            oneh_sb = cpool.tile([128, 32 * 32], bf16, tag="oneh")
            nc.sync.dma_start(oneh_sb[:], oneh_d[:])
            eye_sb = cpool.tile([128, QPC], bf16, tag="eye")
            nc.sync.dma_start(eye_sb[:], eye_d[:])

            # ---- projections (bf16 in, f32 psum; QpT f32 / KpT bf16 out)
            qp_sb = projpool.tile([128, NQL], f32, tag="qp")
            qp_ps = projps.tile([128, 512], f32, tag="projps")
            for dc in range(4):
                nc.tensor.matmul(
                    qp_ps[:, :NQL], wq_sb[dc][:], qt_sb[dc][:],
                    start=(dc == 0), stop=(dc == 3),
                )
            nc.vector.tensor_copy(qp_sb[:], qp_ps[:, :NQL])

            kp_sb = projpool.tile([128, KSUM2], bf16, tag="kp")
            nkc = (KSUM2 + 511) // 512
            for kc in range(nkc):
                c0 = kc * 512
                cw = min(512, KSUM2 - c0)
                kp_ps = projps.tile([128, 512], f32, tag="projps")
                for dc in range(4):
                    nc.tensor.matmul(
                        kp_ps[:, :cw], wk_sb[dc][:], kt_sb[dc][:, c0 : c0 + cw],
                        start=(dc == 0), stop=(dc == 3),
                    )
                nc.vector.tensor_copy(kp_sb[:, c0 : c0 + cw], kp_ps[:, :cw])

            # ---- main phase: per q-block (128 queries = 2 batch groups)
            for qb in range(2):
                bs = [2 * qb, 2 * qb + 1]
                blockmax = max(L2[b] for b in bs)
                sc_ps = scorps.tile([128, blockmax], f32, tag="scores")

                # scores: tanh(Qp[q]+Kp) reduced against wv via one-hot
                # matmuls into ping-ponged 32-row col-groups
                for gi, b in enumerate(bs):
                    kpad = L2[b]
                    koff = int(OFF2[b])
                    chunk = CH[b]
                    for c in range(QPC // chunk):
                        s_t = spool.tile([128, SMAX], bf16, tag="s")
                        for j in range(chunk):
                            q = b * QPC + c * chunk + j  # core-local query idx
                            nc.vector.tensor_scalar_add(
                                s_t[:, j * kpad : (j + 1) * kpad],
                                kp_sb[:, koff : koff + kpad],
                                qp_sb[:, q : q + 1],
                            )
                        fd = chunk * kpad
                        t_t = tpool.tile([128, SMAX], bf16, tag="t")
                        nc.scalar.activation(t_t[:, :fd], s_t[:, :fd], Tanh)
                        for j in range(chunk):
                            jj = c * chunk + j       # 0..63 within group
                            row = gi * QPC + _pi(jj)  # psum row
                            cg = row // 32            # col-group 0..3
                            i32 = row % 32            # one-hot index
                            first = jj in (0, 1)
                            last = jj in (62, 63)
                            n = blockmax if first else kpad
                            nc.tensor.matmul(
                                sc_ps[cg * 32 : cg * 32 + 32, :n],
                                oneh_sb[:, i32 * 32 : i32 * 32 + 32],
                                t_t[:, j * kpad : j * kpad + n],
                                start=first, stop=last,
                                tile_position=(0, cg * 32),
                            )

                # softmax + P@V per 64-row group
                o_ps = ops.tile([128, DV], f32, tag="ops")
                o_sb = opool.tile([128, DV], f32, tag="osb")
                for gi, b in enumerate(bs):
                    kpadv = KPV[b]
                    koffv = int(OFFV[b])
                    lb = L[b]
                    r0, r1 = gi * QPC, gi * QPC + QPC
                    nmx = statpool.tile([128, 1], f32, tag="nmx")
                    nc.vector.reduce_max(
                        nmx[r0:r1, :], sc_ps[r0:r1, :lb], axis=AX, negate=True
                    )
                    p_t = ppool.tile([128, 512], bf16, tag="p")
                    ssum = statpool.tile([128, 1], f32, tag="ssum")
                    nc.scalar.activation(
                        p_t[r0:r1, :lb], sc_ps[r0:r1, :lb], Exp,
                        bias=nmx[r0:r1, :], accum_out=ssum[r0:r1, :],
                    )
                    if lb < kpadv:
                        nc.vector.memset(p_t[r0:r1, lb:kpadv], 0.0)
                    rs = statpool.tile([128, 1], f32, tag="rs")
                    nc.vector.reciprocal(rs[r0:r1, :], ssum[r0:r1, :])
                    for kc in range(kpadv // 128):
                        wt_ps = tps.tile([128, QPC], bf16, tag="wtps")
                        nc.tensor.transpose(
                            wt_ps[:],
                            p_t[r0:r1, kc * 128 : (kc + 1) * 128],
                            eye_sb[r0:r1, :],
                        )
                        wt_sb = statpool.tile([128, QPC], bf16, tag="wtsb")
                        nc.vector.tensor_copy(wt_sb[:], wt_ps[:])
                        nc.tensor.matmul(
                            o_ps[r0:r1, :],
                            wt_sb[:],
                            v_sb[(koffv + kc * 128) // 128][:],
                            start=(kc == 0), stop=(kc == kpadv // 128 - 1),
                        )
                    nc.scalar.activation(
                        o_sb[r0:r1, :], o_ps[r0:r1, :], Copy, scale=rs[r0:r1, :]
                    )
                nc.sync.dma_start(out_d[qb * 128 : (qb + 1) * 128, :], o_sb[:])

    nc.compile()
    return nc


def _install_profile_hook():
    """Register the NTFF profile hook that this container's antenv lacks,
    so run_bass_kernel_spmd(trace=True) can report exec_time_ns."""
    import types

    import antenv

    try:
        import antenv.axon_hooks  # noqa: F401
        return
    except ImportError:
        pass
    try:
        from trn_agent_boot.trn_boot import _ntff_profile_via_ctypes
    except ImportError:
        return
    hook = _ntff_profile_via_ctypes("/opt/axon/libaxon_pjrt.so")
    m = types.ModuleType("antenv.axon_hooks")
    m.get_axon_ntff_profile_hook = lambda: hook
    m.set_axon_ntff_profile_hook = lambda h: None
    sys.modules["antenv.axon_hooks"] = m
    antenv.axon_hooks = m


def kernel(Q, K, V, Wq, Wk, wv, valid_lens):
    global LAST_EXEC_NS
    import ml_dtypes
    from concourse.bass_utils import run_bass_kernel_spmd

    bfnp = ml_dtypes.bfloat16
    Q = np.asarray(Q, dtype=np.float32)
    K = np.asarray(K, dtype=np.float32)
    V = np.asarray(V, dtype=np.float32)
    Wq = np.asarray(Wq, dtype=np.float32)
    Wk = np.asarray(Wk, dtype=np.float32)
    wv = np.asarray(wv, dtype=np.float32)

    L, L2, KPV, CH = _plan(valid_lens)
    nc = _build_program(L, L2, KPV, CH)

    # shared tensors
    kt = np.ascontiguousarray(
        np.concatenate([K[b, : L2[b], :] for b in range(B)], axis=0).T
    ).astype(bfnp)
    v16 = np.ascontiguousarray(
        np.concatenate([V[b, : KPV[b], :] for b in range(B)], axis=0)
    ).astype(bfnp)
    oneh3 = np.zeros((H, 32, 32), dtype=bfnp)
    oneh3[:, np.arange(32), np.arange(32)] = wv[:, None].astype(bfnp)
    oneh = oneh3.reshape(H, 32 * 32)
    eye = np.concatenate(
        [np.eye(QPC, dtype=bfnp)] * (128 // QPC), axis=0
    )

    in_maps = []
    for c in range(NCORES):
        qloc = np.concatenate(
            [Q[b, c * QPC : (c + 1) * QPC, :] for b in range(B)], axis=0
        )  # (256, 512)
        in_maps.append(
            {
                "qt": np.ascontiguousarray(qloc.T).astype(bfnp),
                "kt": kt,
                "v": v16,
                "wq": Wq.astype(bfnp),
                "wk": Wk.astype(bfnp),
                "oneh": oneh,
                "eye": eye,
            }
        )

    trace = os.environ.get("KERNEL_PROFILE", "0") == "1"
    if trace:
        _install_profile_hook()
    res = run_bass_kernel_spmd(nc, in_maps, list(range(NCORES)), trace=trace)
    LAST_EXEC_NS = res.exec_time_ns
    LAST_RESULT["res"] = res

    # device row for (batch, query j): q-block b//2, group b%2, ping-pong row
    devrow = np.empty((B, QPC), dtype=np.int64)
    for b in range(B):
        for j in range(QPC):
            devrow[b, j] = (b // 2) * 128 + (b % 2) * QPC + _pi(j)

    out = np.empty((B, NQ, DV), dtype=np.float32)
    for c in range(NCORES):
        o = np.asarray(res.results[c]["out"])
        for b in range(B):
            out[b, c * QPC : (c + 1) * QPC, :] = o[devrow[b], :]
    return out


# revision 13
# speedup vs baseline: 1.1908x; 1.1908x over previous
import os
import sys

import numpy as np

sys.path.insert(0, "/opt/trn_rl_repo")

# Problem constants (nn_AdditiveAttention): hardcoded per spec.
B, NQ, NK, D, DV, H = 4, 512, 512, 512, 512, 128
NCORES = 8
QPC = NQ // NCORES  # queries contributed by each batch to each core (64)
SMAX = 6144         # max s/t pipeline tile free dim (per partition)

LAST_EXEC_NS = None
LAST_RESULT = {}


def _plan(valid_lens):
    L = [int(x) for x in np.asarray(valid_lens).reshape(-1)]
    L2 = [min(NK, -(-l // 2) * 2) for l in L]       # add/tanh/scores extent
    KPV = [min(NK, -(-l // 128) * 128) for l in L]  # PV (128-aligned) extent
    CH = {}
    for b in range(B):
        c = 32
        while c * L2[b] > SMAX:
            c //= 2
        CH[b] = c
    return L, L2, KPV, CH


def _build_program(L, L2, KPV, CH):
    """Build the SPMD Bass program. All cores run this one program;
    per-core data differences come only through in_maps."""
    import concourse.bacc as bacc
    import concourse.mybir as mybir
    from concourse.tile import TileContext

    f32 = mybir.dt.float32
    bf16 = mybir.dt.bfloat16
    OFF2 = np.concatenate([[0], np.cumsum(L2)]).astype(int)
    OFFV = np.concatenate([[0], np.cumsum(KPV)]).astype(int)
    KSUM2 = int(OFF2[-1])
    KSUMV = int(OFFV[-1])
    NQL = B * QPC  # local queries per core (256)

    nc = bacc.Bacc("TRN2", target_bir_lowering=False, debug=False)

    qt_d = nc.dram_tensor("qt", [D, NQL], bf16, kind="ExternalInput")
    kt_d = nc.dram_tensor("kt", [D, KSUM2], bf16, kind="ExternalInput")
    v_d = nc.dram_tensor("v", [KSUMV, DV], bf16, kind="ExternalInput")
    wq_d = nc.dram_tensor("wq", [D, H], bf16, kind="ExternalInput")
    wk_d = nc.dram_tensor("wk", [D, H], bf16, kind="ExternalInput")
    oneh_d = nc.dram_tensor("oneh", [H, QPC * QPC], bf16, kind="ExternalInput")
    eye_d = nc.dram_tensor("eye", [QPC, QPC], bf16, kind="ExternalInput")
    out_d = nc.dram_tensor("out", [NQL, DV], f32, kind="ExternalOutput")

    Tanh = mybir.ActivationFunctionType.Tanh
    Exp = mybir.ActivationFunctionType.Exp
    Copy = mybir.ActivationFunctionType.Copy
    AX = mybir.AxisListType.X

    with TileContext(nc) as tc:
        with (
            tc.tile_pool(name="const", bufs=1) as cpool,
            tc.tile_pool(name="proj", bufs=1) as projpool,
            tc.tile_pool(name="s", bufs=3) as spool,
            tc.tile_pool(name="t", bufs=3) as tpool,
            tc.tile_pool(name="p", bufs=2) as ppool,
            tc.tile_pool(name="stat", bufs=4) as statpool,
            tc.tile_pool(name="osb", bufs=2) as opool,
        ):
            # ---- load constants
            qt_sb = []
            kt_sb = []
            wq_sb = []
            wk_sb = []
            for i in range(4):
                t = cpool.tile([128, NQL], bf16, tag=f"qt{i}")
                nc.sync.dma_start(t[:], qt_d.rearrange("(n p) m -> n p m", p=128)[i])
                qt_sb.append(t)
                t = cpool.tile([128, KSUM2], bf16, tag=f"kt{i}")
                nc.sync.dma_start(t[:], kt_d.rearrange("(n p) m -> n p m", p=128)[i])
                kt_sb.append(t)
                t = cpool.tile([128, H], bf16, tag=f"wq{i}")
                nc.sync.dma_start(t[:], wq_d.rearrange("(n p) m -> n p m", p=128)[i])
                wq_sb.append(t)
                t = cpool.tile([128, H], bf16, tag=f"wk{i}")
                nc.sync.dma_start(t[:], wk_d.rearrange("(n p) m -> n p m", p=128)[i])
                wk_sb.append(t)
            v_sb = []
            for i in range(KSUMV // 128):
                t = cpool.tile([128, DV], bf16, tag=f"v{i}")
                nc.sync.dma_start(t[:], v_d.rearrange("(n p) m -> n p m", p=128)[i])
                v_sb.append(t)
            oneh_sb = cpool.tile([128, QPC * QPC], bf16, tag="oneh")
            nc.sync.dma_start(oneh_sb[:], oneh_d[:])
            eye_sb = cpool.tile([QPC, QPC], bf16, tag="eye")
            nc.sync.dma_start(eye_sb[:], eye_d[:])

            # ---- projections (bf16 in, f32 psum; QpT f32 / KpT bf16 out)
            qp_sb = projpool.tile([128, NQL], f32, tag="qp")
            kp_sb = projpool.tile([128, KSUM2], bf16, tag="kp")
            with tc.tile_pool(name="pps", bufs=2, space="PSUM") as projps:
                qp_ps = projps.tile([128, 512], f32, tag="projps")
                for dc in range(4):
                    nc.tensor.matmul(
                        qp_ps[:, :NQL], wq_sb[dc][:], qt_sb[dc][:],
                        start=(dc == 0), stop=(dc == 3),
                    )
                nc.vector.tensor_copy(qp_sb[:], qp_ps[:, :NQL])

                nkc = (KSUM2 + 511) // 512
                for kc in range(nkc):
                    c0 = kc * 512
                    cw = min(512, KSUM2 - c0)
                    kp_ps = projps.tile([128, 512], f32, tag="projps")
                    for dc in range(4):
                        nc.tensor.matmul(
                            kp_ps[:, :cw], wk_sb[dc][:], kt_sb[dc][:, c0 : c0 + cw],
                            start=(dc == 0), stop=(dc == 3),
                        )
                    nc.vector.tensor_copy(kp_sb[:, c0 : c0 + cw], kp_ps[:, :cw])

            # ---- main phase: one 64-query group per batch, own PSUM tiles;
            # softmax/PV of group g-1 emitted after the first chunk of g so
            # no engine stalls at a group boundary.
            with (
                tc.tile_pool(name="sps", bufs=3, space="PSUM") as scorps,
                tc.tile_pool(name="ops", bufs=2, space="PSUM") as ops,
                tc.tile_pool(name="tps", bufs=2, space="PSUM") as tps,
            ):
                sc_tiles = {}

                def softmax_pv(b):
                    kpadv = KPV[b]
                    koffv = int(OFFV[b])
                    lb = L[b]
                    sc_ps = sc_tiles.pop(b)
                    nmx = statpool.tile([128, 1], f32, tag="nmx")
                    nc.vector.reduce_max(
                        nmx[:QPC, :], sc_ps[:QPC, :lb], axis=AX, negate=True
                    )
                    p_t = ppool.tile([QPC, 512], bf16, tag="p")
                    ssum = statpool.tile([128, 1], f32, tag="ssum")
                    nc.scalar.activation(
                        p_t[:, :lb], sc_ps[:QPC, :lb], Exp,
                        bias=nmx[:QPC, :], accum_out=ssum[:QPC, :],
                    )
                    if lb < kpadv:
                        nc.vector.memset(p_t[:, lb:kpadv], 0.0)
                    rs = statpool.tile([128, 1], f32, tag="rs")
                    nc.vector.reciprocal(rs[:QPC, :], ssum[:QPC, :])
                    o_ps = ops.tile([QPC, DV], f32, tag="ops")
                    for kc in range(kpadv // 128):
                        wt_ps = tps.tile([128, QPC], bf16, tag="wtps")
                        nc.tensor.transpose(
                            wt_ps[:],
                            p_t[:, kc * 128 : (kc + 1) * 128],
                            eye_sb[:],
                        )
                        wt_sb = statpool.tile([128, QPC], bf16, tag="wtsb")
                        nc.vector.tensor_copy(wt_sb[:], wt_ps[:])
                        nc.tensor.matmul(
                            o_ps[:],
                            wt_sb[:],
                            v_sb[(koffv + kc * 128) // 128][:],
                            start=(kc == 0), stop=(kc == kpadv // 128 - 1),
                        )
                    o_sb = opool.tile([QPC, DV], f32, tag="osb")
                    nc.scalar.activation(o_sb[:], o_ps[:], Copy, scale=rs[:QPC, :])
                    nc.sync.dma_start(out_d[b * QPC : (b + 1) * QPC, :], o_sb[:])

                for b in range(B):
                    kpad = L2[b]
                    koff = int(OFF2[b])
                    chunk = CH[b]
                    sc_ps = scorps.tile([128, kpad], f32, tag="scores")
                    sc_tiles[b] = sc_ps
                    for c in range(QPC // chunk):
                        s_t = spool.tile([128, SMAX], bf16, tag="s")
                        for j in range(chunk):
                            q = b * QPC + c * chunk + j  # core-local query idx
                            nc.vector.tensor_scalar_add(
                                s_t[:, j * kpad : (j + 1) * kpad],
                                kp_sb[:, koff : koff + kpad],
                                qp_sb[:, q : q + 1],
                            )
                        fd = chunk * kpad
                        t_t = tpool.tile([128, SMAX], bf16, tag="t")
                        nc.scalar.activation(t_t[:, :fd], s_t[:, :fd], Tanh)
                        for j in range(chunk):
                            jj = c * chunk + j  # 0..63 within group
                            nc.tensor.matmul(
                                sc_ps[:QPC, :kpad],
                                oneh_sb[:, jj * QPC : (jj + 1) * QPC],
                                t_t[:, j * kpad : j * kpad + kpad],
                                start=(jj == 0), stop=(jj == QPC - 1),
                            )
                        if c == 0 and b > 0:
                            softmax_pv(b - 1)
                softmax_pv(B - 1)

    nc.compile()
    return nc


def _install_profile_hook():
    """Register the NTFF profile hook that this container's antenv lacks,
    so run_bass_kernel_spmd(trace=True) can report exec_time_ns."""
    import types

    import antenv

    try:
        import antenv.axon_hooks  # noqa: F401
        return
    except ImportError:
        pass
    try:
        from trn_agent_boot.trn_boot import _ntff_profile_via_ctypes
    except ImportError:
        return
    hook = _ntff_profile_via_ctypes("/opt/axon/libaxon_pjrt.so")
    m = types.ModuleType("antenv.axon_hooks")
    m.get_axon_ntff_profile_hook = lambda: hook
    m.set_axon_ntff_profile_hook = lambda h: None
    sys.modules["antenv.axon_hooks"] = m
    antenv.axon_hooks = m


def kernel(Q, K, V, Wq, Wk, wv, valid_lens):
    global LAST_EXEC_NS
    import ml_dtypes
    from concourse.bass_utils import run_bass_kernel_spmd

    bfnp = ml_dtypes.bfloat16
    Q = np.asarray(Q, dtype=np.float32)
    K = np.asarray(K, dtype=np.float32)
    V = np.asarray(V, dtype=np.float32)
    Wq = np.asarray(Wq, dtype=np.float32)
    Wk = np.asarray(Wk, dtype=np.float32)
    wv = np.asarray(wv, dtype=np.float32)

    L, L2, KPV, CH = _plan(valid_lens)
    nc = _build_program(L, L2, KPV, CH)

    # shared tensors
    kt = np.ascontiguousarray(
        np.concatenate([K[b, : L2[b], :] for b in range(B)], axis=0).T
    ).astype(bfnp)
    v16 = np.ascontiguousarray(
        np.concatenate([V[b, : KPV[b], :] for b in range(B)], axis=0)
    ).astype(bfnp)
    oneh3 = np.zeros((H, QPC, QPC), dtype=bfnp)
    oneh3[:, np.arange(QPC), np.arange(QPC)] = wv[:, None].astype(bfnp)
    oneh = oneh3.reshape(H, QPC * QPC)
    eye = np.eye(QPC, dtype=bfnp)

    in_maps = []
    for c in range(NCORES):
        qloc = np.concatenate(
            [Q[b, c * QPC : (c + 1) * QPC, :] for b in range(B)], axis=0
        )  # (256, 512)
        in_maps.append(
            {
                "qt": np.ascontiguousarray(qloc.T).astype(bfnp),
                "kt": kt,
                "v": v16,
                "wq": Wq.astype(bfnp),
                "wk": Wk.astype(bfnp),
                "oneh": oneh,
                "eye": eye,
            }
        )

    trace = os.environ.get("KERNEL_PROFILE", "0") == "1"
    if trace:
        _install_profile_hook()
    res = run_bass_kernel_spmd(nc, in_maps, list(range(NCORES)), trace=trace)
    LAST_EXEC_NS = res.exec_time_ns
    LAST_RESULT["res"] = res

    out = np.empty((B, NQ, DV), dtype=np.float32)
    for c in range(NCORES):
        o = np.asarray(res.results[c]["out"])
        for b in range(B):
            out[b, c * QPC : (c + 1) * QPC, :] = o[b * QPC : (b + 1) * QPC, :]
    return out


# revision 15
# speedup vs baseline: 1.1943x; 1.0029x over previous
import os
import sys

import numpy as np

sys.path.insert(0, "/opt/trn_rl_repo")

# Problem constants (nn_AdditiveAttention): hardcoded per spec.
B, NQ, NK, D, DV, H = 4, 512, 512, 512, 512, 128
NCORES = 8
QPC = NQ // NCORES  # queries contributed by each batch to each core (64)
SMAX = 6144         # max s/t pipeline tile free dim (per partition)

LAST_EXEC_NS = None
LAST_RESULT = {}


def _plan(valid_lens):
    L = [int(x) for x in np.asarray(valid_lens).reshape(-1)]
    L2 = [min(NK, -(-l // 2) * 2) for l in L]       # add/tanh/scores extent
    KPV = [min(NK, -(-l // 128) * 128) for l in L]  # PV (128-aligned) extent
    CH = {}
    for b in range(B):
        c = 32
        while c * L2[b] > SMAX:
            c //= 2
        CH[b] = c
    return L, L2, KPV, CH


def _build_program(L, L2, KPV, CH):
    """Build the SPMD Bass program. All cores run this one program;
    per-core data differences come only through in_maps."""
    import concourse.bacc as bacc
    import concourse.mybir as mybir
    from concourse.tile import TileContext

    f32 = mybir.dt.float32
    bf16 = mybir.dt.bfloat16
    OFF2 = np.concatenate([[0], np.cumsum(L2)]).astype(int)
    OFFV = np.concatenate([[0], np.cumsum(KPV)]).astype(int)
    KSUM2 = int(OFF2[-1])
    KSUMV = int(OFFV[-1])
    NQL = B * QPC  # local queries per core (256)

    nc = bacc.Bacc("TRN2", target_bir_lowering=False, debug=False)

    qt_d = nc.dram_tensor("qt", [D, NQL], bf16, kind="ExternalInput")
    kt_d = nc.dram_tensor("kt", [D, KSUM2], bf16, kind="ExternalInput")
    v_d = nc.dram_tensor("v", [KSUMV, DV], bf16, kind="ExternalInput")
    wq_d = nc.dram_tensor("wq", [D, H], bf16, kind="ExternalInput")
    wk_d = nc.dram_tensor("wk", [D, H], bf16, kind="ExternalInput")
    oneh_d = nc.dram_tensor("oneh", [H, QPC * QPC], bf16, kind="ExternalInput")
    eye_d = nc.dram_tensor("eye", [QPC, QPC], bf16, kind="ExternalInput")
    out_d = nc.dram_tensor("out", [NQL, DV], f32, kind="ExternalOutput")

    Tanh = mybir.ActivationFunctionType.Tanh
    Exp = mybir.ActivationFunctionType.Exp
    Copy = mybir.ActivationFunctionType.Copy
    AX = mybir.AxisListType.X

    with TileContext(nc) as tc:
        with (
            tc.tile_pool(name="const", bufs=1) as cpool,
            tc.tile_pool(name="proj", bufs=1) as projpool,
            tc.tile_pool(name="s", bufs=3) as spool,
            tc.tile_pool(name="t", bufs=3) as tpool,
            tc.tile_pool(name="p", bufs=2) as ppool,
            tc.tile_pool(name="stat", bufs=6) as statpool,
            tc.tile_pool(name="osb", bufs=2) as opool,
        ):
            # ---- load constants (critical-path DMAs first; V tiles are
            # emitted later, inside the main loop, to keep the head short)
            kt_sb = [cpool.tile([128, KSUM2], bf16, tag=f"kt{i}", name=f"kt{i}") for i in range(4)]
            wk_sb = [cpool.tile([128, H], bf16, tag=f"wk{i}", name=f"wk{i}") for i in range(4)]
            qt_sb = [cpool.tile([128, NQL], bf16, tag=f"qt{i}", name=f"qt{i}") for i in range(4)]
            wq_sb = [cpool.tile([128, H], bf16, tag=f"wq{i}", name=f"wq{i}") for i in range(4)]
            for i in range(4):
                nc.sync.dma_start(kt_sb[i][:], kt_d.rearrange("(n p) m -> n p m", p=128)[i])
                nc.sync.dma_start(wk_sb[i][:], wk_d.rearrange("(n p) m -> n p m", p=128)[i])
            for i in range(4):
                nc.sync.dma_start(qt_sb[i][:], qt_d.rearrange("(n p) m -> n p m", p=128)[i])
                nc.sync.dma_start(wq_sb[i][:], wq_d.rearrange("(n p) m -> n p m", p=128)[i])
            oneh_sb = cpool.tile([128, QPC * QPC], bf16, tag="oneh")
            nc.sync.dma_start(oneh_sb[:], oneh_d[:])
            eye_sb = cpool.tile([QPC, QPC], bf16, tag="eye")
            nc.sync.dma_start(eye_sb[:], eye_d[:])
            v_sb = [cpool.tile([128, DV], bf16, tag=f"v{i}", name=f"v{i}") for i in range(KSUMV // 128)]

            def load_v():
                for i in range(KSUMV // 128):
                    nc.sync.dma_start(
                        v_sb[i][:], v_d.rearrange("(n p) m -> n p m", p=128)[i]
                    )

            # ---- projections (bf16 in, f32 psum; QpT f32 / KpT bf16 out)
            qp_sb = projpool.tile([128, NQL], f32, tag="qp")
            kp_sb = [
                projpool.tile([128, L2[b]], bf16, tag=f"kp{b}", name=f"kp{b}")
                for b in range(B)
            ]
            with tc.tile_pool(name="pps", bufs=2, space="PSUM") as projps:
                qp_ps = projps.tile([128, 512], f32, tag="projps")
                for dc in range(4):
                    nc.tensor.matmul(
                        qp_ps[:, :NQL], wq_sb[dc][:], qt_sb[dc][:],
                        start=(dc == 0), stop=(dc == 3),
                    )
                nc.vector.tensor_copy(qp_sb[:], qp_ps[:, :NQL])

                for b in range(B):
                    c0 = int(OFF2[b])
                    cw = L2[b]
                    kp_ps = projps.tile([128, 512], f32, tag="projps")
                    for dc in range(4):
                        nc.tensor.matmul(
                            kp_ps[:, :cw], wk_sb[dc][:], kt_sb[dc][:, c0 : c0 + cw],
                            start=(dc == 0), stop=(dc == 3),
                        )
                    nc.vector.tensor_copy(kp_sb[b][:, :], kp_ps[:, :cw])

            # ---- main phase: one 64-query group per batch, own PSUM tiles;
            # softmax/PV of group g-1 emitted after the first chunk of g so
            # no engine stalls at a group boundary.
            with (
                tc.tile_pool(name="sps", bufs=3, space="PSUM") as scorps,
                tc.tile_pool(name="ops", bufs=2, space="PSUM") as ops,
                tc.tile_pool(name="tps", bufs=2, space="PSUM") as tps,
            ):
                sc_tiles = {}

                def softmax_pv(b):
                    kpadv = KPV[b]
                    koffv = int(OFFV[b])
                    lb = L[b]
                    sc_ps = sc_tiles.pop(b)
                    nmx = statpool.tile([128, 1], f32, tag="nmx")
                    nc.vector.reduce_max(
                        nmx[:QPC, :], sc_ps[:QPC, :lb], axis=AX, negate=True
                    )
                    p_t = ppool.tile([QPC, 512], bf16, tag="p")
                    ssum = statpool.tile([128, 1], f32, tag="ssum")
                    nc.scalar.activation(
                        p_t[:, :lb], sc_ps[:QPC, :lb], Exp,
                        bias=nmx[:QPC, :], accum_out=ssum[:QPC, :],
                    )
                    if lb < kpadv:
                        nc.vector.memset(p_t[:, lb:kpadv], 0.0)
                    rs = statpool.tile([128, 1], f32, tag="rs")
                    nc.vector.reciprocal(rs[:QPC, :], ssum[:QPC, :])
                    o_ps = ops.tile([QPC, DV], f32, tag="ops")
                    for kc in range(kpadv // 128):
                        wt_ps = tps.tile([128, QPC], bf16, tag="wtps")
                        nc.tensor.transpose(
                            wt_ps[:],
                            p_t[:, kc * 128 : (kc + 1) * 128],
                            eye_sb[:],
                        )
                        wt_sb = statpool.tile([128, QPC], bf16, tag="wtsb")
                        nc.vector.tensor_copy(wt_sb[:], wt_ps[:])
                        nc.tensor.matmul(
                            o_ps[:],
                            wt_sb[:],
                            v_sb[(koffv + kc * 128) // 128][:],
                            start=(kc == 0), stop=(kc == kpadv // 128 - 1),
                        )
                    o_sb = opool.tile([QPC, DV], f32, tag="osb")
                    nc.scalar.activation(o_sb[:], o_ps[:], Copy, scale=rs[:QPC, :])
                    nc.sync.dma_start(out_d[b * QPC : (b + 1) * QPC, :], o_sb[:])

                for b in range(B):
                    kpad = L2[b]
                    koff = int(OFF2[b])
                    chunk = CH[b]
                    sc_ps = scorps.tile([128, kpad], f32, tag="scores")
                    sc_tiles[b] = sc_ps
                    for c in range(QPC // chunk):
                        s_t = spool.tile([128, SMAX], bf16, tag="s")
                        for j in range(chunk):
                            q = b * QPC + c * chunk + j  # core-local query idx
                            nc.vector.tensor_scalar_add(
                                s_t[:, j * kpad : (j + 1) * kpad],
                                kp_sb[b][:, :],
                                qp_sb[:, q : q + 1],
                            )
                        fd = chunk * kpad
                        t_t = tpool.tile([128, SMAX], bf16, tag="t")
                        nc.scalar.activation(t_t[:, :fd], s_t[:, :fd], Tanh)
                        for j in range(chunk):
                            jj = c * chunk + j  # 0..63 within group
                            nc.tensor.matmul(
                                sc_ps[:QPC, :kpad],
                                oneh_sb[:, jj * QPC : (jj + 1) * QPC],
                                t_t[:, j * kpad : j * kpad + kpad],
                                start=(jj == 0), stop=(jj == QPC - 1),
                            )
                        if c == 0 and b == 1:
                            load_v()
                        if c == 0 and b > 0:
                            softmax_pv(b - 1)
                softmax_pv(B - 1)

    nc.compile()
    return nc


def _install_profile_hook():
    """Register the NTFF profile hook that this container's antenv lacks,
    so run_bass_kernel_spmd(trace=True) can report exec_time_ns."""
    import types

    import antenv

    try:
        import antenv.axon_hooks  # noqa: F401
        return
    except ImportError:
        pass
    try:
        from trn_agent_boot.trn_boot import _ntff_profile_via_ctypes
    except ImportError:
        return
    hook = _ntff_profile_via_ctypes("/opt/axon/libaxon_pjrt.so")
    m = types.ModuleType("antenv.axon_hooks")
    m.get_axon_ntff_profile_hook = lambda: hook
    m.set_axon_ntff_profile_hook = lambda h: None
    sys.modules["antenv.axon_hooks"] = m
    antenv.axon_hooks = m


def kernel(Q, K, V, Wq, Wk, wv, valid_lens):
    global LAST_EXEC_NS
    import ml_dtypes
    from concourse.bass_utils import run_bass_kernel_spmd

    bfnp = ml_dtypes.bfloat16
    Q = np.asarray(Q, dtype=np.float32)
    K = np.asarray(K, dtype=np.float32)
    V = np.asarray(V, dtype=np.float32)
    Wq = np.asarray(Wq, dtype=np.float32)
    Wk = np.asarray(Wk, dtype=np.float32)
    wv = np.asarray(wv, dtype=np.float32)

    L, L2, KPV, CH = _plan(valid_lens)
    nc = _build_program(L, L2, KPV, CH)

    # shared tensors
    kt = np.ascontiguousarray(
        np.concatenate([K[b, : L2[b], :] for b in range(B)], axis=0).T
    ).astype(bfnp)
    v16 = np.ascontiguousarray(
        np.concatenate([V[b, : KPV[b], :] for b in range(B)], axis=0)
    ).astype(bfnp)
    oneh3 = np.zeros((H, QPC, QPC), dtype=bfnp)
    oneh3[:, np.arange(QPC), np.arange(QPC)] = wv[:, None].astype(bfnp)
    oneh = oneh3.reshape(H, QPC * QPC)
    eye = np.eye(QPC, dtype=bfnp)

    in_maps = []
    for c in range(NCORES):
        qloc = np.concatenate(
            [Q[b, c * QPC : (c + 1) * QPC, :] for b in range(B)], axis=0
        )  # (256, 512)
        in_maps.append(
            {
                "qt": np.ascontiguousarray(qloc.T).astype(bfnp),
                "kt": kt,
                "v": v16,
                "wq": Wq.astype(bfnp),
                "wk": Wk.astype(bfnp),
                "oneh": oneh,
                "eye": eye,
            }
        )

    trace = os.environ.get("KERNEL_PROFILE", "0") == "1"
    if trace:
        _install_profile_hook()
    res = run_bass_kernel_spmd(nc, in_maps, list(range(NCORES)), trace=trace)
    LAST_EXEC_NS = res.exec_time_ns
    LAST_RESULT["res"] = res

    out = np.empty((B, NQ, DV), dtype=np.float32)
    for c in range(NCORES):
        o = np.asarray(res.results[c]["out"])
        for b in range(B):
            out[b, c * QPC : (c + 1) * QPC, :] = o[b * QPC : (b + 1) * QPC, :]
    return out


# revision 16
# speedup vs baseline: 1.2496x; 1.0463x over previous
import os
import sys

import numpy as np

sys.path.insert(0, "/opt/trn_rl_repo")

# Problem constants (nn_AdditiveAttention): hardcoded per spec.
B, NQ, NK, D, DV, H = 4, 512, 512, 512, 512, 128
NCORES = 8
QPC = NQ // NCORES  # queries contributed by each batch to each core (64)
SMAX = 6144         # max s/t pipeline tile free dim (per partition)

LAST_EXEC_NS = None
LAST_RESULT = {}


def _plan(valid_lens):
    L = [int(x) for x in np.asarray(valid_lens).reshape(-1)]
    L2 = [min(NK, -(-l // 2) * 2) for l in L]       # add/tanh/scores extent
    KPV = [min(NK, -(-l // 128) * 128) for l in L]  # PV (128-aligned) extent
    CH = {}
    for b in range(B):
        c = 32
        while c * L2[b] > SMAX:
            c //= 2
        chunks = []
        if b == 0:
            # ramp: small first chunks so the ACT/PE pipeline lights up early
            r = 8
            left = QPC
            while left > 0:
                step = min(r, c, left)
                chunks.append(step)
                left -= step
                r *= 2
        else:
            chunks = [c] * (QPC // c)
        CH[b] = chunks
    return L, L2, KPV, CH


def _build_program(L, L2, KPV, CH):
    """Build the SPMD Bass program. All cores run this one program;
    per-core data differences come only through in_maps."""
    import concourse.bacc as bacc
    import concourse.mybir as mybir
    from concourse.tile import TileContext

    f32 = mybir.dt.float32
    bf16 = mybir.dt.bfloat16
    OFF2 = np.concatenate([[0], np.cumsum(L2)]).astype(int)
    OFFV = np.concatenate([[0], np.cumsum(KPV)]).astype(int)
    KSUM2 = int(OFF2[-1])
    KSUMV = int(OFFV[-1])
    NQL = B * QPC  # local queries per core (256)

    nc = bacc.Bacc("TRN2", target_bir_lowering=False, debug=False)

    qt_d = nc.dram_tensor("qt", [D, NQL], bf16, kind="ExternalInput")
    kt_d = nc.dram_tensor("kt", [D, KSUM2], bf16, kind="ExternalInput")
    v_d = nc.dram_tensor("v", [KSUMV, DV], bf16, kind="ExternalInput")
    wq_d = nc.dram_tensor("wq", [D, H], bf16, kind="ExternalInput")
    wk_d = nc.dram_tensor("wk", [D, H], bf16, kind="ExternalInput")
    oneh_d = nc.dram_tensor("oneh", [H, QPC * QPC], bf16, kind="ExternalInput")
    eye_d = nc.dram_tensor("eye", [QPC, QPC], bf16, kind="ExternalInput")
    out_d = nc.dram_tensor("out", [NQL, DV], f32, kind="ExternalOutput")

    Tanh = mybir.ActivationFunctionType.Tanh
    Exp = mybir.ActivationFunctionType.Exp
    Copy = mybir.ActivationFunctionType.Copy
    AX = mybir.AxisListType.X

    with TileContext(nc) as tc:
        with (
            tc.tile_pool(name="const", bufs=1) as cpool,
            tc.tile_pool(name="proj", bufs=1) as projpool,
            tc.tile_pool(name="s", bufs=3) as spool,
            tc.tile_pool(name="t", bufs=3) as tpool,
            tc.tile_pool(name="p", bufs=2) as ppool,
            tc.tile_pool(name="stat", bufs=6) as statpool,
            tc.tile_pool(name="osb", bufs=2) as opool,
        ):
            # ---- load constants (critical-path DMAs first; V tiles are
            # emitted later, inside the main loop, to keep the head short)
            kt_sb = [cpool.tile([128, KSUM2], bf16, tag=f"kt{i}", name=f"kt{i}") for i in range(4)]
            wk_sb = [cpool.tile([128, H], bf16, tag=f"wk{i}", name=f"wk{i}") for i in range(4)]
            qt_sb = [cpool.tile([128, NQL], bf16, tag=f"qt{i}", name=f"qt{i}") for i in range(4)]
            wq_sb = [cpool.tile([128, H], bf16, tag=f"wq{i}", name=f"wq{i}") for i in range(4)]
            for i in range(4):
                eng = nc.sync if i % 2 == 0 else nc.gpsimd
                eng.dma_start(kt_sb[i][:], kt_d.rearrange("(n p) m -> n p m", p=128)[i])
                eng.dma_start(wk_sb[i][:], wk_d.rearrange("(n p) m -> n p m", p=128)[i])
            for i in range(4):
                eng = nc.sync if i % 2 == 0 else nc.gpsimd
                eng.dma_start(qt_sb[i][:], qt_d.rearrange("(n p) m -> n p m", p=128)[i])
                eng.dma_start(wq_sb[i][:], wq_d.rearrange("(n p) m -> n p m", p=128)[i])
            oneh_sb = cpool.tile([128, QPC * QPC], bf16, tag="oneh")
            nc.gpsimd.dma_start(oneh_sb[:], oneh_d[:])
            eye_sb = cpool.tile([QPC, QPC], bf16, tag="eye")
            nc.sync.dma_start(eye_sb[:], eye_d[:])
            v_sb = [cpool.tile([128, DV], bf16, tag=f"v{i}", name=f"v{i}") for i in range(KSUMV // 128)]

            def load_v():
                for i in range(KSUMV // 128):
                    nc.sync.dma_start(
                        v_sb[i][:], v_d.rearrange("(n p) m -> n p m", p=128)[i]
                    )

            # ---- projections (bf16 in, f32 psum; QpT f32 / KpT bf16 out)
            qp_sb = projpool.tile([128, NQL], f32, tag="qp")
            kp_sb = [
                projpool.tile([128, L2[b]], bf16, tag=f"kp{b}", name=f"kp{b}")
                for b in range(B)
            ]
            with tc.tile_pool(name="pps", bufs=2, space="PSUM") as projps:
                qp_ps = projps.tile([128, 512], f32, tag="projps")
                for dc in range(4):
                    nc.tensor.matmul(
                        qp_ps[:, :NQL], wq_sb[dc][:], qt_sb[dc][:],
                        start=(dc == 0), stop=(dc == 3),
                    )
                nc.vector.tensor_copy(qp_sb[:], qp_ps[:, :NQL])

                for b in range(B):
                    c0 = int(OFF2[b])
                    cw = L2[b]
                    kp_ps = projps.tile([128, 512], f32, tag="projps")
                    for dc in range(4):
                        nc.tensor.matmul(
                            kp_ps[:, :cw], wk_sb[dc][:], kt_sb[dc][:, c0 : c0 + cw],
                            start=(dc == 0), stop=(dc == 3),
                        )
                    nc.vector.tensor_copy(kp_sb[b][:, :], kp_ps[:, :cw])

            # ---- main phase: one 64-query group per batch, own PSUM tiles;
            # softmax/PV of group g-1 emitted after the first chunk of g so
            # no engine stalls at a group boundary.
            with (
                tc.tile_pool(name="sps", bufs=3, space="PSUM") as scorps,
                tc.tile_pool(name="ops", bufs=2, space="PSUM") as ops,
                tc.tile_pool(name="tps", bufs=2, space="PSUM") as tps,
            ):
                sc_tiles = {}
                pending = []

                def softmax_stages(b):
                    """Return softmax/PV of batch b as 4 stages to drip-feed
                    between the next batch's score chunks (hides cross-engine
                    dependency-chain latency behind streaming work)."""
                    kpadv = KPV[b]
                    koffv = int(OFFV[b])
                    lb = L[b]
                    sc_ps = sc_tiles.pop(b)
                    box = {}

                    def s1():
                        nmx = statpool.tile([128, 1], f32, tag="nmx", name="nmx")
                        nc.vector.reduce_max(
                            nmx[:QPC, :], sc_ps[:QPC, :lb], axis=AX, negate=True
                        )
                        box["nmx"] = nmx

                    def s2():
                        p_t = ppool.tile([QPC, 512], bf16, tag="p", name="p_t")
                        ssum = statpool.tile([128, 1], f32, tag="ssum", name="ssum")
                        nc.scalar.activation(
                            p_t[:, :lb], sc_ps[:QPC, :lb], Exp,
                            bias=box["nmx"][:QPC, :], accum_out=ssum[:QPC, :],
                        )
                        if lb < kpadv:
                            nc.vector.memset(p_t[:, lb:kpadv], 0.0)
                        box["p_t"] = p_t
                        box["ssum"] = ssum

                    def s3():
                        rs = statpool.tile([128, 1], f32, tag="rs", name="rs")
                        nc.vector.reciprocal(rs[:QPC, :], box["ssum"][:QPC, :])
                        box["rs"] = rs
                        o_ps = ops.tile([QPC, DV], f32, tag="ops", name="o_ps")
                        for kc in range(kpadv // 128):
                            wt_ps = tps.tile([128, QPC], bf16, tag="wtps", name="wt_ps")
                            nc.tensor.transpose(
                                wt_ps[:],
                                box["p_t"][:, kc * 128 : (kc + 1) * 128],
                                eye_sb[:],
                            )
                            wt_sb = statpool.tile([128, QPC], bf16, tag="wtsb", name="wt_sb")
                            nc.vector.tensor_copy(wt_sb[:], wt_ps[:])
                            nc.tensor.matmul(
                                o_ps[:],
                                wt_sb[:],
                                v_sb[(koffv + kc * 128) // 128][:],
                                start=(kc == 0), stop=(kc == kpadv // 128 - 1),
                            )
                        box["o_ps"] = o_ps

                    def s4():
                        o_sb = opool.tile([QPC, DV], f32, tag="osb", name="o_sb")
                        nc.scalar.activation(
                            o_sb[:], box["o_ps"][:], Copy, scale=box["rs"][:QPC, :]
                        )
                        nc.sync.dma_start(
                            out_d[b * QPC : (b + 1) * QPC, :], o_sb[:]
                        )

                    return [s1, s2, s3, s4]

                for b in range(B):
                    kpad = L2[b]
                    sc_ps = scorps.tile([128, kpad], f32, tag="scores", name="sc_ps")
                    sc_tiles[b] = sc_ps
                    qbase = 0
                    for c, chunk in enumerate(CH[b]):
                        s_t = spool.tile([128, SMAX], bf16, tag="s", name="s_t")
                        for j in range(chunk):
                            q = b * QPC + qbase + j  # core-local query idx
                            nc.vector.tensor_scalar_add(
                                s_t[:, j * kpad : (j + 1) * kpad],
                                kp_sb[b][:, :],
                                qp_sb[:, q : q + 1],
                            )
                        fd = chunk * kpad
                        t_t = tpool.tile([128, SMAX], bf16, tag="t", name="t_t")
                        nc.scalar.activation(t_t[:, :fd], s_t[:, :fd], Tanh)
                        for j in range(chunk):
                            jj = qbase + j  # 0..63 within group
                            nc.tensor.matmul(
                                sc_ps[:QPC, :kpad],
                                oneh_sb[:, jj * QPC : (jj + 1) * QPC],
                                t_t[:, j * kpad : j * kpad + kpad],
                                start=(jj == 0), stop=(jj == QPC - 1),
                            )
                        qbase += chunk
                        if c == 0 and b == 1:
                            load_v()
                        if c == 0 and b > 0:
                            pending.extend(softmax_stages(b - 1))
                        if pending:
                            pending.pop(0)()
                pending.extend(softmax_stages(B - 1))
                while pending:
                    pending.pop(0)()

    nc.compile()
    return nc


def _install_profile_hook():
    """Register the NTFF profile hook that this container's antenv lacks,
    so run_bass_kernel_spmd(trace=True) can report exec_time_ns."""
    import types

    import antenv

    try:
        import antenv.axon_hooks  # noqa: F401
        return
    except ImportError:
        pass
    try:
        from trn_agent_boot.trn_boot import _ntff_profile_via_ctypes
    except ImportError:
        return
    hook = _ntff_profile_via_ctypes("/opt/axon/libaxon_pjrt.so")
    m = types.ModuleType("antenv.axon_hooks")
    m.get_axon_ntff_profile_hook = lambda: hook
    m.set_axon_ntff_profile_hook = lambda h: None
    sys.modules["antenv.axon_hooks"] = m
    antenv.axon_hooks = m


def kernel(Q, K, V, Wq, Wk, wv, valid_lens):
    global LAST_EXEC_NS
    import ml_dtypes
    from concourse.bass_utils import run_bass_kernel_spmd

    bfnp = ml_dtypes.bfloat16
    Q = np.asarray(Q, dtype=np.float32)
    K = np.asarray(K, dtype=np.float32)
    V = np.asarray(V, dtype=np.float32)
    Wq = np.asarray(Wq, dtype=np.float32)
    Wk = np.asarray(Wk, dtype=np.float32)
    wv = np.asarray(wv, dtype=np.float32)

    L, L2, KPV, CH = _plan(valid_lens)
    nc = _build_program(L, L2, KPV, CH)

    # shared tensors
    kt = np.ascontiguousarray(
        np.concatenate([K[b, : L2[b], :] for b in range(B)], axis=0).T
    ).astype(bfnp)
    v16 = np.ascontiguousarray(
        np.concatenate([V[b, : KPV[b], :] for b in range(B)], axis=0)
    ).astype(bfnp)
    oneh3 = np.zeros((H, QPC, QPC), dtype=bfnp)
    oneh3[:, np.arange(QPC), np.arange(QPC)] = wv[:, None].astype(bfnp)
    oneh = oneh3.reshape(H, QPC * QPC)
    eye = np.eye(QPC, dtype=bfnp)

    in_maps = []
    for c in range(NCORES):
        qloc = np.concatenate(
            [Q[b, c * QPC : (c + 1) * QPC, :] for b in range(B)], axis=0
        )  # (256, 512)
        in_maps.append(
            {
                "qt": np.ascontiguousarray(qloc.T).astype(bfnp),
                "kt": kt,
                "v": v16,
                "wq": Wq.astype(bfnp),
                "wk": Wk.astype(bfnp),
                "oneh": oneh,
                "eye": eye,
            }
        )

    trace = os.environ.get("KERNEL_PROFILE", "0") == "1"
    if trace:
        _install_profile_hook()
    res = run_bass_kernel_spmd(nc, in_maps, list(range(NCORES)), trace=trace)
    LAST_EXEC_NS = res.exec_time_ns
    LAST_RESULT["res"] = res

    out = np.empty((B, NQ, DV), dtype=np.float32)
    for c in range(NCORES):
        o = np.asarray(res.results[c]["out"])
        for b in range(B):
            out[b, c * QPC : (c + 1) * QPC, :] = o[b * QPC : (b + 1) * QPC, :]
    return out
